# revision 1
# baseline (speedup 1.0000x reference)
"""Trainium2 Bass kernel for nn_Node_Transformation.

Computes, for row n:
    out[n] = emb_weight[node_type[n]]                 if node_type[n] != item_id
             x[n] @ W.T + b                           if node_type[n] == item_id

Equivalent formulation used on device (exact, float-add commutative):
    table2       = emb_weight with row item_id replaced by b
    out[n]       = table2[node_type[n]] + mask[n] * (x[n] @ W.T)

Sharding: data-parallel over N across 8 NeuronCores. Weights/table replicated.
Per-core rows are laid out "partition-major": global (in-shard) row index
r = p*F + f  for partition p in [0,128) and tile column f in [0,F).
"""

import os
import numpy as np

import concourse.bass as bass
import concourse.bacc as bacc
import concourse.mybir as mybir
from concourse.tile import TileContext
from concourse.bass import IndirectOffsetOnAxis
from concourse.bass_utils import run_bass_kernel_spmd
from concourse.masks import make_identity

# ---- problem constants (hardcoded per contest contract) ----
N = 500000
IN_CH = 256
HID = 128
NUM_T = 8
NCORES = 8
P = 128
NSH = N // NCORES          # 62500 real rows per core
F = (NSH + P - 1) // P     # 489 tile columns
PAD = P * F                # 62592 padded rows per core

_CACHE = {}


def _ensure_axon_profile_hook():
    """bass_utils' trace path imports antenv.axon_hooks, which this image
    lacks. Register an equivalent module backed by the axon PJRT .so so
    trace=True (or BASS_TRACE=1) works instead of crashing."""
    try:
        import antenv.axon_hooks  # noqa: F401
        return
    except ImportError:
        pass
    import sys
    import types

    hook = None
    try:
        from trn_agent_boot.trn_boot import _ntff_profile_via_ctypes

        hook = _ntff_profile_via_ctypes("/opt/axon/libaxon_pjrt.so")
    except Exception:
        hook = None
    mod = types.ModuleType("antenv.axon_hooks")
    mod.get_axon_ntff_profile_hook = lambda: hook
    mod.set_axon_ntff_profile_hook = lambda h: None
    sys.modules["antenv.axon_hooks"] = mod
    try:
        import antenv

        antenv.axon_hooks = mod
    except ImportError:
        pass


def _build(item: int) -> bass.Bass:
    nc = bacc.Bacc("TRN2")
    f32 = mybir.dt.float32
    i32 = mybir.dt.int32

    x_d = nc.dram_tensor("x", [PAD, IN_CH], f32, kind="ExternalInput")
    nt_d = nc.dram_tensor("nt", [PAD], i32, kind="ExternalInput")
    t2_d = nc.dram_tensor("table2", [NUM_T, HID], f32, kind="ExternalInput")
    wt_d = nc.dram_tensor("wt", [IN_CH, HID], f32, kind="ExternalInput")
    out_d = nc.dram_tensor("out", [PAD, HID], f32, kind="ExternalOutput")

    x_v = x_d[:].rearrange("(p f) c -> p f c", p=P)     # [128, F, 256]
    nt_v = nt_d[:].rearrange("(p f) -> p f", p=P)       # [128, F]
    out_v = out_d[:].rearrange("(p f) h -> p f h", p=P) # [128, F, 128]

    with TileContext(nc) as tc:
        with (
            tc.tile_pool(name="singles", bufs=1) as singles,
            tc.tile_pool(name="xp", bufs=4) as xpool,
            tc.tile_pool(name="tp", bufs=4) as tpool,
            tc.tile_pool(name="op", bufs=4) as opool,
            tc.tile_pool(name="ps", bufs=2, space="PSUM") as pspool,
        ):
            ident = singles.tile([P, P], f32)
            make_identity(nc, ident)

            wt_s = singles.tile([P, 2, HID], f32)
            nc.sync.dma_start(out=wt_s[:], in_=wt_d[:].rearrange("(c k) h -> k c h", c=2))

            nt_all = singles.tile([P, F], i32)
            nc.sync.dma_start(out=nt_all[:], in_=nt_v)
            ntf = singles.tile([P, F], f32)
            nc.vector.tensor_copy(ntf[:], nt_all[:])
            eq_all = singles.tile([P, F], f32)
            nc.vector.tensor_scalar(
                out=eq_all[:], in0=ntf[:], scalar1=float(item), scalar2=None,
                op0=mybir.AluOpType.is_equal,
            )

            for f in range(F):
                x_t = xpool.tile([P, IN_CH], f32, tag="x")
                nc.sync.dma_start(out=x_t[:], in_=x_v[:, f, :])
                xm = xpool.tile([P, IN_CH], f32, tag="xm")
                nc.vector.tensor_tensor(
                    out=xm[:], in0=x_t[:],
                    in1=eq_all[:, f : f + 1].to_broadcast([P, IN_CH]),
                    op=mybir.AluOpType.mult,
                )

                pt = pspool.tile([P, P], f32, tag="pt")
                pt2 = pspool.tile([P, P], f32, tag="pt2")
                nc.tensor.transpose(pt[:], xm[:, 0:P], ident[:])
                nc.tensor.transpose(pt2[:], xm[:, P : 2 * P], ident[:])
                xt = tpool.tile([P, 2, P], f32, tag="xt")
                nc.vector.tensor_copy(xt[:, 0, :], pt[:])
                nc.vector.tensor_copy(xt[:, 1, :], pt2[:])

                lin = pspool.tile([P, HID], f32, tag="lin")
                nc.tensor.matmul(out=lin[:], lhsT=xt[:, 0, :], rhs=wt_s[:, 0, :],
                                 start=True, stop=False)
                nc.tensor.matmul(out=lin[:], lhsT=xt[:, 1, :], rhs=wt_s[:, 1, :],
                                 start=False, stop=True)

                emb_t = opool.tile([P, HID], f32, tag="emb")
                nc.gpsimd.indirect_dma_start(
                    out=emb_t[:], out_offset=None, in_=t2_d[:],
                    in_offset=IndirectOffsetOnAxis(ap=nt_all[:, f : f + 1], axis=0),
                )
                o_t = opool.tile([P, HID], f32, tag="o")
                nc.vector.tensor_tensor(out=o_t[:], in0=emb_t[:], in1=lin[:],
                                        op=mybir.AluOpType.add)
                nc.scalar.dma_start(out=out_v[:, f, :], in_=o_t[:])
    nc.compile()
    return nc


def _prepare(inputs):
    x = np.asarray(inputs["x"], dtype=np.float32)
    nt = np.asarray(inputs["node_type"]).astype(np.int32)
    item = int(np.asarray(inputs["item_id"]))
    emb = np.asarray(inputs["emb_weight"], dtype=np.float32)
    W = np.asarray(inputs["W"], dtype=np.float32)
    b = np.asarray(inputs["b"], dtype=np.float32)

    table2 = emb.copy()
    table2[item] = b
    wt = np.ascontiguousarray(W.T)  # [IN_CH, HID]
    pad_val = np.int32((item + 1) % NUM_T)  # never selected

    in_maps = []
    for c in range(NCORES):
        xp = np.zeros((PAD, IN_CH), np.float32)
        xp[:NSH] = x[c * NSH : (c + 1) * NSH]
        ntp = np.full(PAD, pad_val, np.int32)
        ntp[:NSH] = nt[c * NSH : (c + 1) * NSH]
        in_maps.append({"x": xp, "nt": ntp, "table2": table2, "wt": wt})
    return item, in_maps


def _run(inputs, trace=False):
    _ensure_axon_profile_hook()
    item, in_maps = _prepare(inputs)
    if item not in _CACHE:
        _CACHE[item] = _build(item)
    nc = _CACHE[item]
    res = run_bass_kernel_spmd(nc, in_maps, core_ids=list(range(NCORES)), trace=trace)
    out = np.empty((N, HID), np.float32)
    for c in range(NCORES):
        out[c * NSH : (c + 1) * NSH] = res.results[c]["out"][:NSH]
    return out, res


def kernel(**inputs) -> np.ndarray:
    out, _ = _run(inputs, trace=bool(os.environ.get("KERNEL_TRACE")))
    return out



# revision 6
# speedup vs baseline: 3.2995x; 3.2995x over previous
"""Trainium2 Bass kernel for nn_Node_Transformation.

Reference semantics, for row n:
    out[n] = x[n] @ W.T + b            if node_type[n] == item_id
             emb_weight[node_type[n]]  otherwise

Only ~1/8 of rows take the linear path; every other row is one of 7
constant 128-float vectors. The host-side sharding step therefore groups
each core's rows by node_type (selected rows first, then one contiguous
run per other type, each padded to a 128-row tile boundary). The device
kernel then:
  * reads ONLY the selected rows of x (pre-transposed to [256, S]),
    computes lin = x_sel @ W.T + b via PE-array matmuls, and writes it
    to the head of the output;
  * writes each constant run by repeatedly DMA-ing a replicated
    SBUF-resident tile (built once from the table via a ones-matmul).
The host inverse-permutes device rows back to their original positions.

HBM traffic per core: ~8.3 MB read + ~32.2 MB write (vs ~96 MB for the
dense formulation) -> memory-roofline ~115 us at 358 GB/s.
"""

import os
import numpy as np

import concourse.bass as bass
import concourse.bacc as bacc
import concourse.mybir as mybir
from concourse.tile import TileContext
from concourse.bass_utils import run_bass_kernel_spmd

# ---- problem constants (hardcoded per contest contract) ----
N = 500000
IN_CH = 256
HID = 128
NUM_T = 8
NCORES = 8
P = 128
NSH = N // NCORES          # 62500 rows per core
KT = 4                     # 128-row tiles per write chunk (512 rows, 256 KB)

_CACHE = {}


def _ensure_axon_profile_hook():
    """bass_utils' trace path imports antenv.axon_hooks, which this image
    lacks. Register an equivalent module backed by the axon PJRT .so so
    trace=True (or BASS_TRACE=1) works instead of crashing."""
    try:
        import antenv.axon_hooks  # noqa: F401
        return
    except ImportError:
        pass
    import sys
    import types

    hook = None
    try:
        from trn_agent_boot.trn_boot import _ntff_profile_via_ctypes

        hook = _ntff_profile_via_ctypes("/opt/axon/libaxon_pjrt.so")
    except Exception:
        hook = None
    mod = types.ModuleType("antenv.axon_hooks")
    mod.get_axon_ntff_profile_hook = lambda: hook
    mod.set_axon_ntff_profile_hook = lambda h: None
    sys.modules["antenv.axon_hooks"] = mod
    try:
        import antenv

        antenv.axon_hooks = mod
    except ImportError:
        pass


def _build(S: int, consts: tuple) -> bass.Bass:
    """S: selected-row region size (rows, multiple of KT*128).
    consts: tuple of per-group padded row counts (each a multiple of 128),
    one per non-selected node type, laid out after the first S rows."""
    nc = bacc.Bacc("TRN2")
    f32 = mybir.dt.float32
    ngroups = len(consts)
    pad2 = S + sum(consts)

    xt_d = nc.dram_tensor("xt", [IN_CH, max(S, 1)], f32, kind="ExternalInput")
    wt_d = nc.dram_tensor("wt", [IN_CH, HID], f32, kind="ExternalInput")
    b_d = nc.dram_tensor("b", [1, HID], f32, kind="ExternalInput")
    # each row: the group's constant 128-vector tiled 4x along the free dim
    cb_d = nc.dram_tensor("cb", [max(ngroups, 1), KT * HID], f32,
                          kind="ExternalInput")
    out_d = nc.dram_tensor("out", [pad2, HID], f32, kind="ExternalOutput")

    def out_chunk(r0, ktiles):
        return out_d[r0 : r0 + ktiles * P, :].rearrange("(k p) h -> p k h", p=P)

    with TileContext(nc) as tc:
        with (
            tc.tile_pool(name="singles", bufs=1) as singles,
            tc.tile_pool(name="xp", bufs=4) as xpool,
            tc.tile_pool(name="op", bufs=4) as opool,
            tc.tile_pool(name="ps", bufs=4, space="PSUM") as pspool,
        ):
            ones1 = singles.tile([1, P], f32)
            nc.vector.memset(ones1[:], 1.0)
            b_s = singles.tile([1, HID], f32)
            nc.sync.dma_start(out=b_s[:], in_=b_d[:])
            wt_s = singles.tile([P, 2, HID], f32)
            nc.sync.dma_start(
                out=wt_s[:], in_=wt_d[:].rearrange("(two c) h -> c two h", two=2)
            )

            # Build the replicated constant tiles: [128, KT, HID] each, all
            # partitions equal to the group's table row.
            const_s = []
            if ngroups:
                for t in range(ngroups):
                    stage_t = singles.tile([1, KT * HID], f32)
                    nc.sync.dma_start(out=stage_t[:], in_=cb_d[t : t + 1, :])
                    pc = pspool.tile([P, KT * HID], f32, tag="pc")
                    nc.tensor.matmul(out=pc[:], lhsT=ones1[:],
                                     rhs=stage_t[:],
                                     start=True, stop=True)
                    ct = singles.tile([P, KT, HID], f32)
                    nc.scalar.copy(
                        ct[:], pc[:].rearrange("p (k h) -> p k h", k=KT)
                    )
                    const_s.append(ct)

            # Linear region: S rows in chunks of KT tiles.
            for g in range(S // (KT * P)):
                c0 = g * KT * P
                xt0 = xpool.tile([P, KT, P], f32, tag="x0")
                xt1 = xpool.tile([P, KT, P], f32, tag="x1")
                nc.sync.dma_start(
                    out=xt0[:],
                    in_=xt_d[0:P, c0 : c0 + KT * P].rearrange(
                        "c (k p) -> c k p", k=KT),
                )
                nc.sync.dma_start(
                    out=xt1[:],
                    in_=xt_d[P : 2 * P, c0 : c0 + KT * P].rearrange(
                        "c (k p) -> c k p", k=KT),
                )
                ps = pspool.tile([P, KT, HID], f32, tag="ps")
                for k in range(KT):
                    nc.tensor.matmul(out=ps[:, k, :], lhsT=xt0[:, k, :],
                                     rhs=wt_s[:, 0, :], start=True, stop=False)
                    nc.tensor.matmul(out=ps[:, k, :], lhsT=xt1[:, k, :],
                                     rhs=wt_s[:, 1, :], start=False, stop=False)
                    nc.tensor.matmul(out=ps[:, k, :], lhsT=ones1[:],
                                     rhs=b_s[:], start=False, stop=True,
                                     skip_group_check=True)
                o_t = opool.tile([P, KT, HID], f32, tag="o")
                nc.scalar.copy(o_t[:], ps[:])
                nc.scalar.dma_start(out=out_chunk(c0, KT), in_=o_t[:])

            # Constant regions: repeated writes of the replicated tiles.
            r0 = S
            for t in range(ngroups):
                tiles = consts[t] // P
                for j in range(tiles // KT):
                    nc.gpsimd.dma_start(out=out_chunk(r0 + j * KT * P, KT),
                                        in_=const_s[t][:])
                tail = tiles % KT
                if tail:
                    nc.gpsimd.dma_start(
                        out=out_chunk(r0 + (tiles - tail) * P, tail),
                        in_=const_s[t][:, 0:tail, :],
                    )
                r0 += consts[t]
    nc.compile()
    return nc


def _round_up(v, m):
    return (v + m - 1) // m * m


def _prepare(inputs):
    x = np.ascontiguousarray(np.asarray(inputs["x"], dtype=np.float32))
    nt = np.asarray(inputs["node_type"]).astype(np.int64).ravel()
    item = int(np.asarray(inputs["item_id"]))
    emb = np.asarray(inputs["emb_weight"], dtype=np.float32)
    b = np.asarray(inputs["b"], dtype=np.float32)
    W = np.asarray(inputs["W"], dtype=np.float32)
    wt = np.ascontiguousarray(W.T)  # [IN_CH, HID]

    const_types = [t for t in range(NUM_T) if t != item]

    sel_idx, grp_idx = [], []
    for c in range(NCORES):
        nt_c = nt[c * NSH : (c + 1) * NSH]
        sel_idx.append(np.flatnonzero(nt_c == item))
        grp_idx.append([np.flatnonzero(nt_c == t) for t in const_types])

    S = _round_up(max(len(s) for s in sel_idx), KT * P)
    consts = tuple(
        _round_up(max(len(grp_idx[c][g]) for c in range(NCORES)), P)
        for g in range(len(const_types))
    )

    cb = np.tile(emb[const_types], (1, KT)) if const_types else \
        np.zeros((1, KT * HID), np.float32)
    b2d = np.ascontiguousarray(b.reshape(1, HID))

    in_maps = []
    for c in range(NCORES):
        xt = np.zeros((IN_CH, max(S, 1)), np.float32)
        si = sel_idx[c]
        if len(si):
            xt[:, : len(si)] = x[c * NSH + si].T
        in_maps.append({"xt": xt, "wt": wt, "b": b2d,
                        "cb": np.ascontiguousarray(cb, np.float32)})
    return S, consts, sel_idx, grp_idx, in_maps


def _run(inputs, trace=False):
    _ensure_axon_profile_hook()
    S, consts, sel_idx, grp_idx, in_maps = _prepare(inputs)
    key = (S, consts)
    if key not in _CACHE:
        _CACHE[key] = _build(S, consts)
    nc = _CACHE[key]
    res = run_bass_kernel_spmd(nc, in_maps, core_ids=list(range(NCORES)),
                               trace=trace)
    out = np.empty((N, HID), np.float32)
    for c in range(NCORES):
        r = res.results[c]["out"]
        out_c = out[c * NSH : (c + 1) * NSH]
        si = sel_idx[c]
        if len(si):
            out_c[si] = r[: len(si)]
        off = S
        for g, gi in enumerate(grp_idx[c]):
            if len(gi):
                out_c[gi] = r[off : off + len(gi)]
            off += consts[g]
    return out, res


def kernel(**inputs) -> np.ndarray:
    out, _ = _run(inputs, trace=bool(os.environ.get("KERNEL_TRACE")))
    return out


# revision 7
# speedup vs baseline: 4.1896x; 1.2698x over previous
"""Trainium2 Bass kernel for nn_Node_Transformation.

Reference semantics, for row n:
    out[n] = x[n] @ W.T + b            if node_type[n] == item_id
             emb_weight[node_type[n]]  otherwise

Only ~1/8 of rows take the linear path; every other row is one of 7
constant 128-float vectors. The host-side sharding step therefore groups
each core's rows by node_type (selected rows first, then one contiguous
run per other type, each padded to a 128-row tile boundary). The device
kernel then:
  * reads ONLY the selected rows of x (pre-transposed to [256, S], cast
    to bf16), computes lin = x_sel @ W.T via PE-array matmuls, adds the
    bias in fp32 while moving PSUM->SBUF, and writes it to the head of
    the output;
  * writes each constant run by repeatedly DMA-ing a replicated
    SBUF-resident tile (built once from the table via a ones-matmul),
    with the writes spread round-robin over the gpsimd/sync/scalar DMA
    queues so the aggregate hits the per-core HBM share.
The host inverse-permutes device rows back to their original positions.

HBM traffic per core: ~4.3 MB read + ~32.6 MB write -> memory-roofline
~103 us at 358 GB/s (vs ~96 MB and ~270 us for the dense formulation).
"""

import os
import numpy as np
import ml_dtypes

import concourse.bass as bass
import concourse.bacc as bacc
import concourse.mybir as mybir
from concourse.tile import TileContext
from concourse.bass_utils import run_bass_kernel_spmd

# ---- problem constants (hardcoded per contest contract) ----
N = 500000
IN_CH = 256
HID = 128
NUM_T = 8
NCORES = 8
P = 128
NSH = N // NCORES          # 62500 rows per core
KT = 4                     # 128-row tiles per write chunk (512 rows, 256 KB)

_CACHE = {}


def _ensure_axon_profile_hook():
    """bass_utils' trace path imports antenv.axon_hooks, which this image
    lacks. Register an equivalent module backed by the axon PJRT .so so
    trace=True (or BASS_TRACE=1) works instead of crashing."""
    try:
        import antenv.axon_hooks  # noqa: F401
        return
    except ImportError:
        pass
    import sys
    import types

    hook = None
    try:
        from trn_agent_boot.trn_boot import _ntff_profile_via_ctypes

        hook = _ntff_profile_via_ctypes("/opt/axon/libaxon_pjrt.so")
    except Exception:
        hook = None
    mod = types.ModuleType("antenv.axon_hooks")
    mod.get_axon_ntff_profile_hook = lambda: hook
    mod.set_axon_ntff_profile_hook = lambda h: None
    sys.modules["antenv.axon_hooks"] = mod
    try:
        import antenv

        antenv.axon_hooks = mod
    except ImportError:
        pass


def _build(S: int, consts: tuple) -> bass.Bass:
    """S: selected-row region size (rows, multiple of KT*128).
    consts: tuple of per-group padded row counts (each a multiple of 128),
    one per non-selected node type, laid out after the first S rows."""
    nc = bacc.Bacc("TRN2")
    f32 = mybir.dt.float32
    bf16 = mybir.dt.bfloat16
    ngroups = len(consts)
    pad2 = S + sum(consts)

    xt_d = nc.dram_tensor("xt", [IN_CH, max(S, 1)], bf16, kind="ExternalInput")
    wt_d = nc.dram_tensor("wt", [IN_CH, HID], bf16, kind="ExternalInput")
    # rows 0..ngroups-1: the group constants; row ngroups: the bias b.
    # Each row is the 128-vector tiled KT times along the free dim.
    cb_d = nc.dram_tensor("cb", [ngroups + 1, KT * HID], f32,
                          kind="ExternalInput")
    out_d = nc.dram_tensor("out", [pad2, HID], f32, kind="ExternalOutput")

    def out_chunk(r0, ktiles):
        return out_d[r0 : r0 + ktiles * P, :].rearrange("(k p) h -> p k h", p=P)

    with TileContext(nc) as tc:
        with (
            tc.tile_pool(name="singles", bufs=1) as singles,
            tc.tile_pool(name="xp", bufs=4) as xpool,
            tc.tile_pool(name="op", bufs=4) as opool,
            tc.tile_pool(name="ps", bufs=4, space="PSUM") as pspool,
        ):
            ones1 = singles.tile([1, P], f32)
            nc.vector.memset(ones1[:], 1.0)
            wt_s = singles.tile([P, 2, HID], bf16)
            nc.sync.dma_start(
                out=wt_s[:], in_=wt_d[:].rearrange("(two c) h -> c two h", two=2)
            )

            # Replicate each cb row across all 128 partitions via a
            # ones-matmul: const tiles for the groups, bias tile last.
            rep_tiles = []
            for t in range(ngroups + 1):
                stage_t = singles.tile([1, KT * HID], f32)
                nc.sync.dma_start(out=stage_t[:], in_=cb_d[t : t + 1, :])
                pc = pspool.tile([P, KT * HID], f32, tag="pc")
                nc.tensor.matmul(out=pc[:], lhsT=ones1[:], rhs=stage_t[:],
                                 start=True, stop=True)
                ct = singles.tile([P, KT, HID], f32)
                nc.scalar.copy(ct[:], pc[:].rearrange("p (k h) -> p k h", k=KT))
                rep_tiles.append(ct)
            const_s, bias_rep = rep_tiles[:ngroups], rep_tiles[ngroups]

            # Linear region: S rows in chunks of KT tiles.
            for g in range(S // (KT * P)):
                c0 = g * KT * P
                xt0 = xpool.tile([P, KT, P], bf16, tag="x0")
                xt1 = xpool.tile([P, KT, P], bf16, tag="x1")
                nc.sync.dma_start(
                    out=xt0[:],
                    in_=xt_d[0:P, c0 : c0 + KT * P].rearrange(
                        "c (k p) -> c k p", k=KT),
                )
                nc.sync.dma_start(
                    out=xt1[:],
                    in_=xt_d[P : 2 * P, c0 : c0 + KT * P].rearrange(
                        "c (k p) -> c k p", k=KT),
                )
                ps = pspool.tile([P, KT, HID], f32, tag="ps")
                for k in range(KT):
                    nc.tensor.matmul(out=ps[:, k, :], lhsT=xt0[:, k, :],
                                     rhs=wt_s[:, 0, :], start=True, stop=False)
                    nc.tensor.matmul(out=ps[:, k, :], lhsT=xt1[:, k, :],
                                     rhs=wt_s[:, 1, :], start=False, stop=True)
                o_t = opool.tile([P, KT, HID], f32, tag="o")
                # PSUM -> SBUF move fused with the fp32 bias add.
                nc.vector.tensor_tensor(out=o_t[:], in0=ps[:], in1=bias_rep[:],
                                        op=mybir.AluOpType.add)
                nc.scalar.dma_start(out=out_chunk(c0, KT), in_=o_t[:])

            # Constant regions: repeated writes of the replicated tiles,
            # spread over three DMA queues.
            queues = [nc.gpsimd, nc.sync, nc.gpsimd, nc.scalar]
            qi = 0
            r0 = S
            for t in range(ngroups):
                tiles = consts[t] // P
                for j in range(tiles // KT):
                    queues[qi % len(queues)].dma_start(
                        out=out_chunk(r0 + j * KT * P, KT), in_=const_s[t][:])
                    qi += 1
                tail = tiles % KT
                if tail:
                    queues[qi % len(queues)].dma_start(
                        out=out_chunk(r0 + (tiles - tail) * P, tail),
                        in_=const_s[t][:, 0:tail, :],
                    )
                    qi += 1
                r0 += consts[t]
    nc.compile()
    return nc


def _round_up(v, m):
    return (v + m - 1) // m * m


def _prepare(inputs):
    x = np.ascontiguousarray(np.asarray(inputs["x"], dtype=np.float32))
    nt = np.asarray(inputs["node_type"]).astype(np.int64).ravel()
    item = int(np.asarray(inputs["item_id"]))
    emb = np.asarray(inputs["emb_weight"], dtype=np.float32)
    b = np.asarray(inputs["b"], dtype=np.float32)
    W = np.asarray(inputs["W"], dtype=np.float32)
    wt = np.ascontiguousarray(W.T.astype(ml_dtypes.bfloat16))  # [IN_CH, HID]

    const_types = [t for t in range(NUM_T) if t != item]

    sel_idx, grp_idx = [], []
    for c in range(NCORES):
        nt_c = nt[c * NSH : (c + 1) * NSH]
        sel_idx.append(np.flatnonzero(nt_c == item))
        grp_idx.append([np.flatnonzero(nt_c == t) for t in const_types])

    S = _round_up(max(len(s) for s in sel_idx), KT * P)
    consts = tuple(
        _round_up(max(len(grp_idx[c][g]) for c in range(NCORES)), P)
        for g in range(len(const_types))
    )

    rows = (np.concatenate([emb[const_types], b.reshape(1, HID)], axis=0)
            if const_types else b.reshape(1, HID))
    cb = np.ascontiguousarray(np.tile(rows, (1, KT)), dtype=np.float32)

    in_maps = []
    for c in range(NCORES):
        xt = np.zeros((IN_CH, max(S, 1)), ml_dtypes.bfloat16)
        si = sel_idx[c]
        if len(si):
            xt[:, : len(si)] = x[c * NSH + si].T.astype(ml_dtypes.bfloat16)
        in_maps.append({"xt": xt, "wt": wt, "cb": cb})
    return S, consts, sel_idx, grp_idx, in_maps


def _run(inputs, trace=False):
    _ensure_axon_profile_hook()
    S, consts, sel_idx, grp_idx, in_maps = _prepare(inputs)
    key = (S, consts)
    if key not in _CACHE:
        _CACHE[key] = _build(S, consts)
    nc = _CACHE[key]
    res = run_bass_kernel_spmd(nc, in_maps, core_ids=list(range(NCORES)),
                               trace=trace)
    out = np.empty((N, HID), np.float32)
    for c in range(NCORES):
        r = res.results[c]["out"]
        out_c = out[c * NSH : (c + 1) * NSH]
        si = sel_idx[c]
        if len(si):
            out_c[si] = r[: len(si)]
        off = S
        for g, gi in enumerate(grp_idx[c]):
            if len(gi):
                out_c[gi] = r[off : off + len(gi)]
            off += consts[g]
    return out, res


def kernel(**inputs) -> np.ndarray:
    out, _ = _run(inputs, trace=bool(os.environ.get("KERNEL_TRACE")))
    return out


# revision 9
# speedup vs baseline: 4.6879x; 1.1190x over previous
"""Trainium2 Bass kernel for nn_Node_Transformation.

Reference semantics, for row n:
    out[n] = x[n] @ W.T + b            if node_type[n] == item_id
             emb_weight[node_type[n]]  otherwise

Only ~1/8 of rows take the linear path; every other row is one of 7
constant 128-float vectors. The host-side sharding step therefore groups
each core's rows by node_type (selected rows first, then one contiguous
run per other type, each padded to a 128-row tile boundary). The device
kernel then:
  * reads ONLY the selected rows of x (pre-transposed to [256, S], cast
    to bf16), computes lin = x_sel @ W.T via PE-array matmuls, adds the
    bias in fp32 while moving PSUM->SBUF, and writes it to the head of
    the output;
  * writes each constant run by repeatedly DMA-ing a replicated
    SBUF-resident tile (built once from the table via a ones-matmul),
    with the writes spread round-robin over the gpsimd/sync/scalar DMA
    queues so the aggregate hits the per-core HBM share.
The host inverse-permutes device rows back to their original positions.

HBM traffic per core: ~4.3 MB read + ~32.6 MB write -> memory-roofline
~103 us at 358 GB/s (vs ~96 MB and ~270 us for the dense formulation).
"""

import os
import numpy as np
import ml_dtypes

import concourse.bass as bass
import concourse.bacc as bacc
import concourse.mybir as mybir
from concourse.tile import TileContext
from concourse.bass_utils import run_bass_kernel_spmd

# ---- problem constants (hardcoded per contest contract) ----
N = 500000
IN_CH = 256
HID = 128
NUM_T = 8
NCORES = 8
P = 128
NSH = N // NCORES          # 62500 rows per core
KT = 4                     # 128-row tiles per PSUM accumulation group
KW = 16                    # tiles per write chunk (2048 rows, 1 MB)

_CACHE = {}


def _ensure_axon_profile_hook():
    """bass_utils' trace path imports antenv.axon_hooks, which this image
    lacks. Register an equivalent module backed by the axon PJRT .so so
    trace=True (or BASS_TRACE=1) works instead of crashing."""
    try:
        import antenv.axon_hooks  # noqa: F401
        return
    except ImportError:
        pass
    import sys
    import types

    hook = None
    try:
        from trn_agent_boot.trn_boot import _ntff_profile_via_ctypes

        hook = _ntff_profile_via_ctypes("/opt/axon/libaxon_pjrt.so")
    except Exception:
        hook = None
    mod = types.ModuleType("antenv.axon_hooks")
    mod.get_axon_ntff_profile_hook = lambda: hook
    mod.set_axon_ntff_profile_hook = lambda h: None
    sys.modules["antenv.axon_hooks"] = mod
    try:
        import antenv

        antenv.axon_hooks = mod
    except ImportError:
        pass


def _build(S: int, consts: tuple) -> bass.Bass:
    """S: selected-row region size (rows, multiple of KT*128).
    consts: tuple of per-group padded row counts (each a multiple of 128),
    one per non-selected node type, laid out after the first S rows."""
    nc = bacc.Bacc("TRN2")
    f32 = mybir.dt.float32
    bf16 = mybir.dt.bfloat16
    ngroups = len(consts)
    pad2 = S + sum(consts)

    xt_d = nc.dram_tensor("xt", [IN_CH, max(S, 1)], bf16, kind="ExternalInput")
    wt_d = nc.dram_tensor("wt", [IN_CH, HID], bf16, kind="ExternalInput")
    # rows 0..ngroups-1: the group constants; row ngroups: the bias b.
    # Each row is the 128-vector tiled KT times along the free dim.
    cb_d = nc.dram_tensor("cb", [ngroups + 1, KT * HID], f32,
                          kind="ExternalInput")
    out_d = nc.dram_tensor("out", [pad2, HID], f32, kind="ExternalOutput")

    def out_chunk(r0, ktiles):
        return out_d[r0 : r0 + ktiles * P, :].rearrange("(k p) h -> p k h", p=P)

    with TileContext(nc) as tc:
        with (
            tc.tile_pool(name="singles", bufs=1) as singles,
            tc.tile_pool(name="xp", bufs=4) as xpool,
            tc.tile_pool(name="op", bufs=4) as opool,
            tc.tile_pool(name="ps", bufs=4, space="PSUM") as pspool,
        ):
            ones1 = singles.tile([1, P], f32)
            nc.vector.memset(ones1[:], 1.0)
            wt_s = singles.tile([P, 2, HID], bf16)
            nc.sync.dma_start(
                out=wt_s[:], in_=wt_d[:].rearrange("(two c) h -> c two h", two=2)
            )

            # Replicate each cb row across all 128 partitions via a
            # ones-matmul. Const tiles are a single [128,1,HID] tile each
            # (fanned out at write time with a stride-0 broadcast source);
            # the bias tile is KT wide to match a PSUM accumulation group.
            const_s = []
            for t in range(ngroups):
                stage_t = singles.tile([1, HID], f32)
                nc.sync.dma_start(out=stage_t[:], in_=cb_d[t : t + 1, 0:HID])
                pc = pspool.tile([P, HID], f32, tag="pc")
                nc.tensor.matmul(out=pc[:], lhsT=ones1[:], rhs=stage_t[:],
                                 start=True, stop=True)
                ct = singles.tile([P, 1, HID], f32)
                nc.scalar.copy(ct[:], pc[:].rearrange("p (k h) -> p k h", k=1))
                const_s.append(ct)
            stage_b = singles.tile([1, KT * HID], f32)
            nc.sync.dma_start(out=stage_b[:], in_=cb_d[ngroups : ngroups + 1, :])
            pb = pspool.tile([P, KT * HID], f32, tag="pc")
            nc.tensor.matmul(out=pb[:], lhsT=ones1[:], rhs=stage_b[:],
                             start=True, stop=True)
            bias_rep = singles.tile([P, KT, HID], f32)
            nc.scalar.copy(bias_rep[:], pb[:].rearrange("p (k h) -> p k h", k=KT))

            # Linear region: S rows in super-groups of KW tiles (one write
            # chunk), each made of KT-tile PSUM accumulation groups.
            stiles = S // P
            for g in range(0, stiles, KW):
                w = min(KW, stiles - g)
                c0 = g * P
                xt0 = xpool.tile([P, KW, P], bf16, tag="x0")
                xt1 = xpool.tile([P, KW, P], bf16, tag="x1")
                nc.sync.dma_start(
                    out=xt0[:, 0:w, :],
                    in_=xt_d[0:P, c0 : c0 + w * P].rearrange(
                        "c (k p) -> c k p", k=w),
                )
                nc.sync.dma_start(
                    out=xt1[:, 0:w, :],
                    in_=xt_d[P : 2 * P, c0 : c0 + w * P].rearrange(
                        "c (k p) -> c k p", k=w),
                )
                o_t = opool.tile([P, KW, HID], f32, tag="o")
                for q in range(0, w, KT):
                    ps = pspool.tile([P, KT, HID], f32, tag="ps")
                    for k in range(KT):
                        nc.tensor.matmul(out=ps[:, k, :],
                                         lhsT=xt0[:, q + k, :],
                                         rhs=wt_s[:, 0, :],
                                         start=True, stop=False)
                        nc.tensor.matmul(out=ps[:, k, :],
                                         lhsT=xt1[:, q + k, :],
                                         rhs=wt_s[:, 1, :],
                                         start=False, stop=True)
                    # PSUM -> SBUF move fused with the fp32 bias add.
                    nc.vector.tensor_tensor(out=o_t[:, q : q + KT, :],
                                            in0=ps[:], in1=bias_rep[:],
                                            op=mybir.AluOpType.add)
                nc.scalar.dma_start(out=out_chunk(c0, w), in_=o_t[:, 0:w, :])

            # Constant regions: broadcast-source writes of the replicated
            # tiles, spread over three DMA queues.
            queues = [nc.gpsimd, nc.sync, nc.scalar]
            qi = 0
            r0 = S
            for t in range(ngroups):
                tiles = consts[t] // P
                j = 0
                while j < tiles:
                    w = min(KW, tiles - j)
                    queues[qi % len(queues)].dma_start(
                        out=out_chunk(r0 + j * P, w),
                        in_=const_s[t][:, 0:1, :].to_broadcast([P, w, HID]),
                    )
                    qi += 1
                    j += w
                r0 += consts[t]
    nc.compile()
    return nc


def _round_up(v, m):
    return (v + m - 1) // m * m


def _prepare(inputs):
    x = np.ascontiguousarray(np.asarray(inputs["x"], dtype=np.float32))
    nt = np.asarray(inputs["node_type"]).astype(np.int64).ravel()
    item = int(np.asarray(inputs["item_id"]))
    emb = np.asarray(inputs["emb_weight"], dtype=np.float32)
    b = np.asarray(inputs["b"], dtype=np.float32)
    W = np.asarray(inputs["W"], dtype=np.float32)
    wt = np.ascontiguousarray(W.T.astype(ml_dtypes.bfloat16))  # [IN_CH, HID]

    const_types = [t for t in range(NUM_T) if t != item]

    sel_idx, grp_idx = [], []
    for c in range(NCORES):
        nt_c = nt[c * NSH : (c + 1) * NSH]
        sel_idx.append(np.flatnonzero(nt_c == item))
        grp_idx.append([np.flatnonzero(nt_c == t) for t in const_types])

    S = _round_up(max(len(s) for s in sel_idx), KT * P)
    consts = tuple(
        _round_up(max(len(grp_idx[c][g]) for c in range(NCORES)), P)
        for g in range(len(const_types))
    )

    rows = (np.concatenate([emb[const_types], b.reshape(1, HID)], axis=0)
            if const_types else b.reshape(1, HID))
    cb = np.ascontiguousarray(np.tile(rows, (1, KT)), dtype=np.float32)

    in_maps = []
    for c in range(NCORES):
        xt = np.zeros((IN_CH, max(S, 1)), ml_dtypes.bfloat16)
        si = sel_idx[c]
        if len(si):
            xt[:, : len(si)] = x[c * NSH + si].T.astype(ml_dtypes.bfloat16)
        in_maps.append({"xt": xt, "wt": wt, "cb": cb})
    return S, consts, sel_idx, grp_idx, in_maps


def _run(inputs, trace=False):
    _ensure_axon_profile_hook()
    S, consts, sel_idx, grp_idx, in_maps = _prepare(inputs)
    key = (S, consts)
    if key not in _CACHE:
        _CACHE[key] = _build(S, consts)
    nc = _CACHE[key]
    res = run_bass_kernel_spmd(nc, in_maps, core_ids=list(range(NCORES)),
                               trace=trace)
    out = np.empty((N, HID), np.float32)
    for c in range(NCORES):
        r = res.results[c]["out"]
        out_c = out[c * NSH : (c + 1) * NSH]
        si = sel_idx[c]
        if len(si):
            out_c[si] = r[: len(si)]
        off = S
        for g, gi in enumerate(grp_idx[c]):
            if len(gi):
                out_c[gi] = r[off : off + len(gi)]
            off += consts[g]
    return out, res


def kernel(**inputs) -> np.ndarray:
    out, _ = _run(inputs, trace=bool(os.environ.get("KERNEL_TRACE")))
    return out
